# revision 4
# baseline (speedup 1.0000x reference)
"""Multi-head attention (B=4, S=2048, E=1024, 16 heads x 64) on 8 Trainium2 cores.

Sharding: core c = 2*b + half handles batch b and heads [8*half, 8*half+8)
(embed slice [512*half, 512*half+512)).  Each core computes its Q/K/V
projections, 8 heads of attention, and a row-parallel out-projection partial
(2048, 1024).  Host unshard: out[b] = partial[2b] + partial[2b+1] + bo.

Per-core device kernel (bf16 matmuls, fp32 accumulation):
  - QT/KT in [d_local, seq] layout (d on partitions) so energy^T = K @ Q^T
    comes out as [k_seq, q_seq] with softmax reductions computable by matmul.
  - The two heads of each m-slice live at partitions 0:64 / 64:128, so their
    energy matmuls carry tile_position (0,0)/(64,0) and run CONCURRENTLY on
    the PE array (row tiling) into separate PSUM banks.
  - softmax without max subtraction (energies ~N(0,1); exp never overflows),
    1/sqrt(64) folded into Wq on the host; exp on the scalar engine.
  - AV matmuls col-tiled: head-even -> PSUM partitions 0:64 (tile (0,0)),
    head-odd -> 64:128 (tile (0,64)) of one shared bank pair - concurrent.
  - softmax denominators via an all-ones [128,32] stationary: 4 concurrent
    M=32 matmuls (tile (0,32j)) accumulate row-sums of exp(energy) for
    (head,q-half) into one PSUM bank.
  - normalization: reciprocal_approx_fast on the whole [128,512] sums bank,
    gpsimd partition_broadcast, multiply into oT; the raw AV output is
    evicted to SBUF first so its PSUM banks recycle without waiting on the
    normalize chain.
"""

import numpy as np
import ml_dtypes

import concourse.bass as bass
import concourse.mybir as mybir
import concourse.tile as tile
import concourse.bacc as bacc
from concourse.bass_utils import run_bass_kernel_spmd

BF16 = mybir.dt.bfloat16
F32 = mybir.dt.float32
NPBF = ml_dtypes.bfloat16

S = 2048          # sequence length
E = 1024          # embed dim
DLOC = 512        # per-core embed slice (8 heads x 64)
HD = 64           # head dim
NHL = 8           # heads per core
KT = E // 128     # 8 contraction tiles for projections
MT = DLOC // 128  # 4 m-tiles of d_local
ST = S // 128     # 16 seq tiles
NCH = S // 512    # 4 seq chunks of 512
EXP = mybir.ActivationFunctionType.Exp
MULT = mybir.AluOpType.mult
ADD = mybir.AluOpType.add


def _build_bass(dump=False):
    nc = bacc.Bacc("TRN2", target_bir_lowering=False, debug=False)

    xqT = nc.dram_tensor("xqT", [E, S], BF16, kind="ExternalInput").ap()
    xkT = nc.dram_tensor("xkT", [E, S], BF16, kind="ExternalInput").ap()
    xvT = nc.dram_tensor("xvT", [E, S], BF16, kind="ExternalInput").ap()
    wq_d = nc.dram_tensor("wq", [E, DLOC], BF16, kind="ExternalInput").ap()
    wk_d = nc.dram_tensor("wk", [E, DLOC], BF16, kind="ExternalInput").ap()
    wv_d = nc.dram_tensor("wv", [E, DLOC], BF16, kind="ExternalInput").ap()
    wo_d = nc.dram_tensor("wo", [DLOC, E], BF16, kind="ExternalInput").ap()
    bq_d = nc.dram_tensor("bq", [128, MT], F32, kind="ExternalInput").ap()
    bk_d = nc.dram_tensor("bk", [128, MT], F32, kind="ExternalInput").ap()
    bv_d = nc.dram_tensor("bv", [1, DLOC], F32, kind="ExternalInput").ap()
    out_d = nc.dram_tensor("out", [S, E], F32, kind="ExternalOutput").ap()

    xq_r = xqT.rearrange("(kt p) s -> p kt s", p=128)
    xk_r = xkT.rearrange("(kt p) s -> p kt s", p=128)
    xv_r = xvT.rearrange("(kt p) s -> p kt s", p=128)

    with tile.TileContext(nc) as tc:
        _kernel_body(tc, nc, xq_r, xk_r, xv_r, wq_d, wk_d, wv_d, wo_d,
                     bq_d, bk_d, bv_d, out_d, dump=dump)
    nc.compile()
    return nc


def _kernel_body(tc, nc, xq_r, xk_r, xv_r, wq_d, wk_d, wv_d, wo_d,
                 bq_d, bk_d, bv_d, out_d, dump=False):
    from contextlib import ExitStack

    with ExitStack() as ctx:
        wpool = ctx.enter_context(tc.tile_pool(name="weights", bufs=1))
        xpool = ctx.enter_context(tc.tile_pool(name="xstream", bufs=3))
        qkv = ctx.enter_context(tc.tile_pool(name="qkv", bufs=1))
        atp = ctx.enter_context(tc.tile_pool(name="attnt", bufs=2))
        smp = ctx.enter_context(tc.tile_pool(name="small", bufs=2))
        orp = ctx.enter_context(tc.tile_pool(name="oraw", bufs=2))
        outp = ctx.enter_context(tc.tile_pool(name="outstage", bufs=3))

        # ---- weights / biases to SBUF ----
        wq_sb = wpool.tile([128, KT, DLOC], BF16)
        wk_sb = wpool.tile([128, KT, DLOC], BF16)
        wv_sb = wpool.tile([128, KT, DLOC], BF16)
        wo_sb = wpool.tile([128, MT, E], BF16)
        bq_sb = wpool.tile([128, MT], F32)
        bk_sb = wpool.tile([128, MT], F32)
        bv_row = wpool.tile([1, DLOC], F32)
        bv_bc = wpool.tile([128, DLOC], F32)
        ones_sb = wpool.tile([128, 32], BF16)
        nc.sync.dma_start(wq_sb[:], wq_d.rearrange("(kt p) m -> p kt m", p=128))
        nc.sync.dma_start(wk_sb[:], wk_d.rearrange("(kt p) m -> p kt m", p=128))
        nc.sync.dma_start(wv_sb[:], wv_d.rearrange("(kt p) m -> p kt m", p=128))
        nc.sync.dma_start(wo_sb[:], wo_d.rearrange("(mt p) e -> p mt e", p=128))
        nc.sync.dma_start(bq_sb[:], bq_d)
        nc.sync.dma_start(bk_sb[:], bk_d)
        nc.sync.dma_start(bv_row[:], bv_d)
        nc.gpsimd.partition_broadcast(bv_bc[:], bv_row[:])
        nc.vector.memset(ones_sb[:], 1.0)

        # ---- persistent per-core tensors ----
        QT_sb = qkv.tile([128, MT, S], BF16)        # [d_loc, seq]
        KT_sb = qkv.tile([128, MT, S], BF16)
        V_sb = qkv.tile([128, ST, NHL, HD], BF16)
        oT_sb = qkv.tile([128, MT, S], BF16)        # attn out^T (lhsT of outproj)

        # PSUM: peA+peB (4 banks) + poP (2) + S (1) + proj (1) = 8 banks.
        pe_pool = ctx.enter_context(tc.tile_pool(name="psum_e", bufs=1, space="PSUM"))
        po_pool = ctx.enter_context(tc.tile_pool(name="psum_o", bufs=1, space="PSUM"))
        ps_pool = ctx.enter_context(tc.tile_pool(name="psum_s", bufs=1, space="PSUM"))
        pj_pool = ctx.enter_context(tc.tile_pool(name="psum_p", bufs=1, space="PSUM"))

        def v_proj_group(nch):
            xv_t = xpool.tile([128, KT, 512], BF16, tag="xs", name="xv_t")
            nc.sync.dma_start(xv_t[:], xv_r[:, :, bass.ts(nch, 512)])
            for stl in range(4):
                st = nch * 4 + stl
                ps = pj_pool.tile([128, 512], F32, tag="proj", name="ps_v")
                for kt in range(KT):
                    nc.tensor.matmul(
                        ps[:], xv_t[:, kt, bass.ts(stl, 128)],
                        wv_sb[:, kt, :], start=(kt == 0), stop=(kt == KT - 1))
                nc.vector.tensor_tensor(
                    V_sb[:, st, :, :],
                    ps[:].rearrange("p (h d) -> p h d", d=HD),
                    bv_bc.rearrange("p (h d) -> p h d", d=HD),
                    ADD)

        def qk_proj_group(ti, m, nch):
            x_r = (xq_r, xk_r)[ti]
            w_sb = (wq_sb, wk_sb)[ti]
            b_sb = (bq_sb, bk_sb)[ti]
            dst = (QT_sb, KT_sb)[ti]
            x_t = xpool.tile([128, KT, 512], BF16, tag="xs", name="x_t")
            nc.sync.dma_start(x_t[:], x_r[:, :, bass.ts(nch, 512)])
            ps = pj_pool.tile([128, 512], F32, tag="proj", name="ps_qk")
            for kt in range(KT):
                nc.tensor.matmul(
                    ps[:], w_sb[:, kt, bass.ts(m, 128)],
                    x_t[:, kt, :], start=(kt == 0), stop=(kt == KT - 1))
            nc.vector.tensor_scalar_add(
                dst[:, m, bass.ts(nch, 512)], ps[:], b_sb[:, m:m + 1])

        # ---- prologue: V fully, then Q/K for m=0 ----
        for nch in range(NCH):
            v_proj_group(nch)
        for ti in range(2):
            for nch in range(NCH):
                qk_proj_group(ti, 0, nch)

        # ---- attention, with next m's Q/K projections woven in ----
        for m in range(MT):
            weave = ([(ti, m + 1, nch) for ti in range(2) for nch in range(NCH)]
                     if m + 1 < MT else [])
            gi = 0
            he, ho = 2 * m, 2 * m + 1          # even/odd head of this m-slice
            for qh in range(2):
                q0 = qh * 1024
                poP = po_pool.tile([128, 1024], F32, tag="po")
                sS = ps_pool.tile([128, 512], F32, tag="S")

                def av_sums_group(pkt, pA, pB):
                    last = pkt == ST - 1
                    for qc in range(2):
                        nc.tensor.matmul(
                            poP[0:64, bass.ts(qc, 512)], V_sb[:, pkt, he, :],
                            pA[:, bass.ts(qc, 512)],
                            start=(pkt == 0), stop=last)
                        nc.tensor.matmul(
                            poP[64:128, bass.ts(qc, 512)], V_sb[:, pkt, ho, :],
                            pB[:, bass.ts(qc, 512)],
                            start=(pkt == 0), stop=last)
                    # denominators: 4 concurrent M=32 col tiles
                    nc.tensor.matmul(sS[0:32, :], ones_sb[:], pA[:, 0:512],
                                     start=(pkt == 0), stop=last,
                                     tile_position=(0, 0))
                    nc.tensor.matmul(sS[32:64, :], ones_sb[:], pB[:, 0:512],
                                     start=(pkt == 0), stop=last,
                                     tile_position=(0, 32))
                    nc.tensor.matmul(sS[64:96, :], ones_sb[:], pA[:, 512:1024],
                                     start=(pkt == 0), stop=last,
                                     tile_position=(0, 64))
                    nc.tensor.matmul(sS[96:128, :], ones_sb[:], pB[:, 512:1024],
                                     start=(pkt == 0), stop=last,
                                     tile_position=(0, 96))

                pending = None
                for kt in range(ST):
                    peA = pe_pool.tile([128, 1024], F32, tag="peA")
                    peB = pe_pool.tile([128, 1024], F32, tag="peB")
                    for qc in range(2):
                        nc.tensor.matmul(
                            peA[:, bass.ts(qc, 512)],
                            KT_sb[0:64, m, bass.ts(kt, 128)],
                            QT_sb[0:64, m, bass.ds(q0 + qc * 512, 512)],
                            start=True, stop=True)
                        nc.tensor.matmul(
                            peB[:, bass.ts(qc, 512)],
                            KT_sb[64:128, m, bass.ts(kt, 128)],
                            QT_sb[64:128, m, bass.ds(q0 + qc * 512, 512)],
                            start=True, stop=True)
                    atA = atp.tile([128, 1024], BF16, tag="atA")
                    atB = atp.tile([128, 1024], BF16, tag="atB")
                    nc.scalar.activation(atA[:], peA[:], EXP)
                    nc.scalar.activation(atB[:], peB[:], EXP)
                    if pending is not None:
                        av_sums_group(*pending)
                    pending = (kt, atA, atB)
                    if kt % 4 == 3 and gi < len(weave):
                        qk_proj_group(*weave[gi])
                        gi += 1
                av_sums_group(*pending)

                # ---- normalize ----
                # evict raw AV output first so poP recycles immediately
                # (two partition-0 tiles: SBUF tensor ops need matching
                # start partitions, PSUM inputs are exempt)
                oraw_e = orp.tile([64, 1024], F32, tag="oraw_e")
                oraw_o = orp.tile([64, 1024], F32, tag="oraw_o")
                nc.vector.tensor_copy(oraw_e[:], poP[0:64, :])
                nc.vector.tensor_copy(oraw_o[:], poP[64:128, :])
                # reciprocals of all 4 (head, q-half) sum blocks in one shot
                rS = smp.tile([128, 1024], F32, tag="rS")
                nc.vector.reciprocal_approx_fast(rS[:, 0:512], sS[:])
                # stage per-head [1,1024] rows at physical partition 0 via
                # SBUF->SBUF DMA (partition_broadcast ucode reads the
                # physical first partition of its input; DVE ops cannot
                # move data across SBUF partitions)
                stg = smp.tile([1, 1024], F32, tag="stg")
                nc.sync.dma_start(rS[0:1, 512:1024], rS[64:65, 0:512])
                nc.sync.dma_start(stg[0:1, 0:512], rS[32:33, 0:512])
                nc.sync.dma_start(stg[0:1, 512:1024], rS[96:97, 0:512])
                bc_e = smp.tile([64, 1024], F32, tag="bce")
                bc_o = smp.tile([64, 1024], F32, tag="bco")
                nc.gpsimd.partition_broadcast(bc_e[:], rS[0:1, :])
                nc.gpsimd.partition_broadcast(bc_o[:], stg[0:1, :])
                nc.vector.tensor_tensor(
                    oT_sb[0:64, m, bass.ds(q0, 1024)],
                    oraw_e[:], bc_e[:], MULT)
                nc.vector.tensor_tensor(
                    oT_sb[64:128, m, bass.ds(q0, 1024)],
                    oraw_o[:], bc_o[:], MULT)

        if dump:
            d_qt = nc.dram_tensor("d_qt", [128, MT, S], BF16, kind="ExternalOutput").ap()
            d_kt = nc.dram_tensor("d_kt", [128, MT, S], BF16, kind="ExternalOutput").ap()
            d_v = nc.dram_tensor("d_v", [128, ST, NHL, HD], BF16, kind="ExternalOutput").ap()
            d_ot = nc.dram_tensor("d_ot", [128, MT, S], BF16, kind="ExternalOutput").ap()
            nc.sync.dma_start(d_qt, QT_sb[:])
            nc.sync.dma_start(d_kt, KT_sb[:])
            nc.sync.dma_start(d_v, V_sb[:])
            nc.sync.dma_start(d_ot, oT_sb[:])

        # ================= phase 3: out-projection =================
        for qt in range(ST):
            ob = outp.tile([128, E], F32, tag="ob")
            for ec in range(2):
                ps = pj_pool.tile([128, 512], F32, tag="proj", name="ps_o")
                for m in range(MT):
                    nc.tensor.matmul(
                        ps[:], oT_sb[:, m, bass.ts(qt, 128)],
                        wo_sb[:, m, bass.ts(ec, 512)],
                        start=(m == 0), stop=(m == MT - 1))
                nc.vector.tensor_copy(ob[:, bass.ts(ec, 512)], ps[:])
            nc.sync.dma_start(out_d[bass.ts(qt, 128), :], ob[:])


_CACHED = {}


def _get_bass():
    if "nc" not in _CACHED:
        _CACHED["nc"] = _build_bass()
    return _CACHED["nc"]


def _prep_core_inputs(c, query, key, value, Wq, bq, Wk, bk, Wv, bv, Wo):
    b, half = c // 2, c % 2
    sl = slice(DLOC * half, DLOC * half + DLOC)
    bq_sl = (bq[sl] * 0.125).astype(np.float32).reshape(MT, 128).T.copy()
    bk_sl = bk[sl].astype(np.float32).reshape(MT, 128).T.copy()
    return {
        "xqT": np.ascontiguousarray(query[b].T).astype(NPBF),
        "xkT": np.ascontiguousarray(key[b].T).astype(NPBF),
        "xvT": np.ascontiguousarray(value[b].T).astype(NPBF),
        "wq": np.ascontiguousarray(Wq[sl, :].T * 0.125).astype(NPBF),
        "wk": np.ascontiguousarray(Wk[sl, :].T).astype(NPBF),
        "wv": np.ascontiguousarray(Wv[sl, :].T).astype(NPBF),
        "wo": np.ascontiguousarray(Wo[:, sl].T).astype(NPBF),
        "bq": np.ascontiguousarray(bq_sl),
        "bk": np.ascontiguousarray(bk_sl),
        "bv": bv[sl].astype(np.float32).reshape(1, DLOC).copy(),
    }


def kernel(query, key, value, Wq, bq, Wk, bk, Wv, bv, Wo, bo,
           trace=False, **run_kwargs):
    query = np.asarray(query, np.float32)
    key = np.asarray(key, np.float32)
    value = np.asarray(value, np.float32)
    Wq, Wk, Wv, Wo = (np.asarray(w, np.float32) for w in (Wq, Wk, Wv, Wo))
    bq, bk, bv, bo = (np.asarray(x, np.float32) for x in (bq, bk, bv, bo))

    nc = _get_bass()
    in_maps = [_prep_core_inputs(c, query, key, value, Wq, bq, Wk, bk, Wv, bv, Wo)
               for c in range(8)]
    res = run_bass_kernel_spmd(nc, in_maps, core_ids=list(range(8)),
                               trace=trace, **run_kwargs)
    _CACHED["last_result"] = res

    B = query.shape[0]
    out = np.empty((B, S, E), np.float32)
    for b in range(B):
        out[b] = res.results[2 * b]["out"] + res.results[2 * b + 1]["out"] + bo
    return out


# revision 6
# speedup vs baseline: 1.1082x; 1.1082x over previous
"""Multi-head attention (B=4, S=2048, E=1024, 16 heads x 64) on 8 Trainium2 cores.

Sharding: core c = 2*b + half handles batch b and heads [8*half, 8*half+8)
(embed slice [512*half, 512*half+512)).  Each core computes its Q/K/V
projections, 8 heads of attention, and a row-parallel out-projection partial
(2048, 1024).  Host unshard: out[b] = partial[2b] + partial[2b+1] + bo.

Per-core device kernel (bf16 matmuls, fp32 accumulation):
  - QT/KT in [d_local, seq] layout (d on partitions) so energy^T = K @ Q^T
    comes out as [k_seq, q_seq] with softmax reductions computable by matmul.
  - softmax without max subtraction (energies ~N(0,1); exp never overflows),
    1/sqrt(64) folded into Wq on the host; exp on the scalar engine.
  - AV matmuls col-tiled: head-even -> PSUM partitions 0:64 (tile (0,0)),
    head-odd -> 64:128 (tile (0,64)) of one shared bank pair - the PE runs
    them concurrently (PSUM write port allows 128 output partitions/cycle).
  - softmax denominators via an all-ones [128,32] stationary: 4 concurrent
    M=32 matmuls (tile (0,32j)) accumulate row-sums of exp(energy) for
    (head,q-half) into one PSUM bank.
  - normalization: reciprocal_approx_fast on the whole [128,512] sums bank,
    gpsimd partition_broadcast, multiply into oT; the raw AV output is
    evicted to SBUF first so its PSUM banks recycle without waiting on the
    normalize chain.
  - nearly all projection work (V/Q/K chunks, first half of the
    out-projection) is woven into the scalar-engine-bound attention loop so
    the PE fills its exp-wait gaps.
"""

import numpy as np
import ml_dtypes

import concourse.bass as bass
import concourse.mybir as mybir
import concourse.tile as tile
import concourse.bacc as bacc
from concourse.bass_utils import run_bass_kernel_spmd

BF16 = mybir.dt.bfloat16
F32 = mybir.dt.float32
NPBF = ml_dtypes.bfloat16

S = 2048          # sequence length
E = 1024          # embed dim
DLOC = 512        # per-core embed slice (8 heads x 64)
HD = 64           # head dim
NHL = 8           # heads per core
KT = E // 128     # 8 contraction tiles for projections
MT = DLOC // 128  # 4 m-tiles of d_local
ST = S // 128     # 16 seq tiles
NCH = S // 512    # 4 seq chunks of 512
EXP = mybir.ActivationFunctionType.Exp
MULT = mybir.AluOpType.mult
ADD = mybir.AluOpType.add


def _build_bass(dump=False):
    nc = bacc.Bacc("TRN2", target_bir_lowering=False, debug=False)

    xqT = nc.dram_tensor("xqT", [E, S], BF16, kind="ExternalInput").ap()
    xkT = nc.dram_tensor("xkT", [E, S], BF16, kind="ExternalInput").ap()
    xvT = nc.dram_tensor("xvT", [E, S], BF16, kind="ExternalInput").ap()
    wq_d = nc.dram_tensor("wq", [E, DLOC], BF16, kind="ExternalInput").ap()
    wk_d = nc.dram_tensor("wk", [E, DLOC], BF16, kind="ExternalInput").ap()
    wv_d = nc.dram_tensor("wv", [E, DLOC], BF16, kind="ExternalInput").ap()
    wo_d = nc.dram_tensor("wo", [DLOC, E], BF16, kind="ExternalInput").ap()
    bq_d = nc.dram_tensor("bq", [128, MT], F32, kind="ExternalInput").ap()
    bk_d = nc.dram_tensor("bk", [128, MT], F32, kind="ExternalInput").ap()
    bv_d = nc.dram_tensor("bv", [1, DLOC], F32, kind="ExternalInput").ap()
    out_d = nc.dram_tensor("out", [S, E], F32, kind="ExternalOutput").ap()

    xq_r = xqT.rearrange("(kt p) s -> p kt s", p=128)
    xk_r = xkT.rearrange("(kt p) s -> p kt s", p=128)
    xv_r = xvT.rearrange("(kt p) s -> p kt s", p=128)

    with tile.TileContext(nc) as tc:
        _kernel_body(tc, nc, xq_r, xk_r, xv_r, wq_d, wk_d, wv_d, wo_d,
                     bq_d, bk_d, bv_d, out_d, dump=dump)
    nc.compile()
    return nc


def _kernel_body(tc, nc, xq_r, xk_r, xv_r, wq_d, wk_d, wv_d, wo_d,
                 bq_d, bk_d, bv_d, out_d, dump=False):
    from contextlib import ExitStack

    with ExitStack() as ctx:
        wpool = ctx.enter_context(tc.tile_pool(name="weights", bufs=1))
        xpool = ctx.enter_context(tc.tile_pool(name="xstream", bufs=3))
        qkv = ctx.enter_context(tc.tile_pool(name="qkv", bufs=1))
        atp = ctx.enter_context(tc.tile_pool(name="attnt", bufs=2))
        smp = ctx.enter_context(tc.tile_pool(name="small", bufs=2))
        orp = ctx.enter_context(tc.tile_pool(name="oraw", bufs=2))
        outp = ctx.enter_context(tc.tile_pool(name="outstage", bufs=3))

        # ---- weights / biases to SBUF (wv first: V projection starts the
        # kernel; wo last: only needed by the out-projection at the end) ----
        wq_sb = wpool.tile([128, KT, DLOC], BF16)
        wk_sb = wpool.tile([128, KT, DLOC], BF16)
        wv_sb = wpool.tile([128, KT, DLOC], BF16)
        wo_sb = wpool.tile([128, MT, E], BF16)
        bq_sb = wpool.tile([128, MT], F32)
        bk_sb = wpool.tile([128, MT], F32)
        bv_row = wpool.tile([1, DLOC], F32)
        bv_bc = wpool.tile([128, DLOC], F32)
        ones_sb = wpool.tile([128, 32], BF16)
        nc.sync.dma_start(wv_sb[:], wv_d.rearrange("(kt p) m -> p kt m", p=128))
        nc.sync.dma_start(bv_row[:], bv_d)
        nc.gpsimd.partition_broadcast(bv_bc[:], bv_row[:])
        nc.vector.memset(ones_sb[:], 1.0)

        # ---- persistent per-core tensors ----
        QT_sb = qkv.tile([128, MT, S], BF16)        # [d_loc, seq]
        KT_sb = qkv.tile([128, MT, S], BF16)
        V_sb = qkv.tile([128, ST, NHL, HD], BF16)
        oT_sb = qkv.tile([128, MT, S], BF16)        # attn out^T (lhsT of outproj)

        # PSUM: peA+peB (4 banks) + poP (2) + S (1) + proj (1) = 8 banks.
        pe_pool = ctx.enter_context(tc.tile_pool(name="psum_e", bufs=1, space="PSUM"))
        po_pool = ctx.enter_context(tc.tile_pool(name="psum_o", bufs=1, space="PSUM"))
        ps_pool = ctx.enter_context(tc.tile_pool(name="psum_s", bufs=1, space="PSUM"))
        pj_pool = ctx.enter_context(tc.tile_pool(name="psum_p", bufs=1, space="PSUM"))

        def v_proj_group(nch):
            xv_t = xpool.tile([128, KT, 512], BF16, tag="xs", name="xv_t")
            nc.sync.dma_start(xv_t[:], xv_r[:, :, bass.ts(nch, 512)])
            for stl in range(4):
                st = nch * 4 + stl
                ps = pj_pool.tile([128, 512], F32, tag="proj", name="ps_v")
                for kt in range(KT):
                    nc.tensor.matmul(
                        ps[:], xv_t[:, kt, bass.ts(stl, 128)],
                        wv_sb[:, kt, :], start=(kt == 0), stop=(kt == KT - 1))
                nc.vector.tensor_tensor(
                    V_sb[:, st, :, :],
                    ps[:].rearrange("p (h d) -> p h d", d=HD),
                    bv_bc.rearrange("p (h d) -> p h d", d=HD),
                    ADD)

        def qk_proj_group(ti, m, nch):
            x_r = (xq_r, xk_r)[ti]
            w_sb = (wq_sb, wk_sb)[ti]
            b_sb = (bq_sb, bk_sb)[ti]
            dst = (QT_sb, KT_sb)[ti]
            x_t = xpool.tile([128, KT, 512], BF16, tag="xs", name="x_t")
            nc.sync.dma_start(x_t[:], x_r[:, :, bass.ts(nch, 512)])
            ps = pj_pool.tile([128, 512], F32, tag="proj", name="ps_qk")
            for kt in range(KT):
                nc.tensor.matmul(
                    ps[:], w_sb[:, kt, bass.ts(m, 128)],
                    x_t[:, kt, :], start=(kt == 0), stop=(kt == KT - 1))
            nc.vector.tensor_scalar_add(
                dst[:, m, bass.ts(nch, 512)], ps[:], b_sb[:, m:m + 1])

        def outproj_group(qt):
            ob = outp.tile([128, E], F32, tag="ob")
            for ec in range(2):
                ps = pj_pool.tile([128, 512], F32, tag="proj", name="ps_o")
                for m in range(MT):
                    nc.tensor.matmul(
                        ps[:], oT_sb[:, m, bass.ts(qt, 128)],
                        wo_sb[:, m, bass.ts(ec, 512)],
                        start=(m == 0), stop=(m == MT - 1))
                nc.vector.tensor_copy(ob[:, bass.ts(ec, 512)], ps[:])
            nc.sync.dma_start(out_d[bass.ts(qt, 128), :], ob[:])

        # ---- prologue: only what the first attention iterations need.
        # q/k weights ride the second HWDGE queue (scalar engine) so they
        # don't serialize behind the x-chunk streams on the sync queue.
        nc.scalar.dma_start(wq_sb[:], wq_d.rearrange("(kt p) m -> p kt m", p=128))
        nc.scalar.dma_start(bq_sb[:], bq_d)
        nc.scalar.dma_start(wk_sb[:], wk_d.rearrange("(kt p) m -> p kt m", p=128))
        nc.scalar.dma_start(bk_sb[:], bk_d)
        v_proj_group(0)
        v_proj_group(1)
        qk_proj_group(0, 0, 0)   # QT m0 q 0:512
        qk_proj_group(0, 0, 1)   # QT m0 q 512:1024
        qk_proj_group(1, 0, 0)   # KT m0 kseq 0:512
        qk_proj_group(1, 0, 1)   # KT m0 kseq 512:1024
        nc.scalar.dma_start(wo_sb[:], wo_d.rearrange("(mt p) e -> p mt e", p=128))

        # ---- weave schedules: one group per odd kt of each (m, qh) pass ----
        def wv_sched(m, qh):
            if m == 0 and qh == 0:
                return [lambda: qk_proj_group(1, 0, 2),
                        lambda: qk_proj_group(1, 0, 3),
                        lambda: v_proj_group(2),
                        lambda: v_proj_group(3),
                        lambda: qk_proj_group(0, 0, 2),
                        lambda: qk_proj_group(0, 0, 3)]
            if m == 0 and qh == 1:
                return [lambda t=ti, c=nch: qk_proj_group(t, 1, c)
                        for ti in range(2) for nch in range(NCH)]
            if m in (1, 2):
                # 4 groups per qh -> 8 per m: projections for m+1
                grps = [(ti, m + 1, nch) for ti in range(2) for nch in range(NCH)]
                half = grps[qh * 4:qh * 4 + 4]
                return [lambda g=g: qk_proj_group(*g) for g in half]
            if m == 3 and qh == 1:
                return [lambda q=qt: outproj_group(q) for qt in range(8)]
            return []

        # ---- attention ----
        for m in range(MT):
            he, ho = 2 * m, 2 * m + 1          # even/odd head of this m-slice
            for qh in range(2):
                q0 = qh * 1024
                weave = wv_sched(m, qh)
                gi = 0
                poP = po_pool.tile([128, 1024], F32, tag="po")
                sS = ps_pool.tile([128, 512], F32, tag="S")

                def av_sums_group(pkt, pA, pB):
                    last = pkt == ST - 1
                    for qc in range(2):
                        nc.tensor.matmul(
                            poP[0:64, bass.ts(qc, 512)], V_sb[:, pkt, he, :],
                            pA[:, bass.ts(qc, 512)],
                            start=(pkt == 0), stop=last)
                        nc.tensor.matmul(
                            poP[64:128, bass.ts(qc, 512)], V_sb[:, pkt, ho, :],
                            pB[:, bass.ts(qc, 512)],
                            start=(pkt == 0), stop=last)
                    # denominators: 4 concurrent M=32 col tiles
                    nc.tensor.matmul(sS[0:32, :], ones_sb[:], pA[:, 0:512],
                                     start=(pkt == 0), stop=last,
                                     tile_position=(0, 0))
                    nc.tensor.matmul(sS[32:64, :], ones_sb[:], pB[:, 0:512],
                                     start=(pkt == 0), stop=last,
                                     tile_position=(0, 32))
                    nc.tensor.matmul(sS[64:96, :], ones_sb[:], pA[:, 512:1024],
                                     start=(pkt == 0), stop=last,
                                     tile_position=(0, 64))
                    nc.tensor.matmul(sS[96:128, :], ones_sb[:], pB[:, 512:1024],
                                     start=(pkt == 0), stop=last,
                                     tile_position=(0, 96))

                pending = None
                for kt in range(ST):
                    peA = pe_pool.tile([128, 1024], F32, tag="peA")
                    peB = pe_pool.tile([128, 1024], F32, tag="peB")
                    for qc in range(2):
                        nc.tensor.matmul(
                            peA[:, bass.ts(qc, 512)],
                            KT_sb[0:64, m, bass.ts(kt, 128)],
                            QT_sb[0:64, m, bass.ds(q0 + qc * 512, 512)],
                            start=True, stop=True)
                    for qc in range(2):
                        nc.tensor.matmul(
                            peB[:, bass.ts(qc, 512)],
                            KT_sb[64:128, m, bass.ts(kt, 128)],
                            QT_sb[64:128, m, bass.ds(q0 + qc * 512, 512)],
                            start=True, stop=True)
                    atA = atp.tile([128, 1024], BF16, tag="atA")
                    atB = atp.tile([128, 1024], BF16, tag="atB")
                    nc.scalar.activation(atA[:], peA[:], EXP)
                    nc.scalar.activation(atB[:], peB[:], EXP)
                    if pending is not None:
                        av_sums_group(*pending)
                    pending = (kt, atA, atB)
                    if kt % 2 == 1 and gi < len(weave):
                        weave[gi]()
                        gi += 1
                av_sums_group(*pending)

                # ---- normalize ----
                # evict raw AV output first so poP recycles immediately
                # (two partition-0 tiles: SBUF tensor ops need matching
                # start partitions, PSUM inputs are exempt)
                oraw_e = orp.tile([64, 1024], F32, tag="oraw_e")
                oraw_o = orp.tile([64, 1024], F32, tag="oraw_o")
                nc.vector.tensor_copy(oraw_e[:], poP[0:64, :])
                nc.vector.tensor_copy(oraw_o[:], poP[64:128, :])
                # reciprocals of all 4 (head, q-half) sum blocks in one shot
                rS = smp.tile([128, 1024], F32, tag="rS")
                nc.vector.reciprocal_approx_fast(rS[:, 0:512], sS[:])
                # stage per-head [1,1024] rows at physical partition 0 via
                # SBUF->SBUF DMA (partition_broadcast ucode reads the
                # physical first partition of its input; DVE ops cannot
                # move data across SBUF partitions)
                stg = smp.tile([1, 1024], F32, tag="stg")
                nc.sync.dma_start(rS[0:1, 512:1024], rS[64:65, 0:512])
                nc.sync.dma_start(stg[0:1, 0:512], rS[32:33, 0:512])
                nc.sync.dma_start(stg[0:1, 512:1024], rS[96:97, 0:512])
                bc_e = smp.tile([64, 1024], F32, tag="bce")
                bc_o = smp.tile([64, 1024], F32, tag="bco")
                nc.gpsimd.partition_broadcast(bc_e[:], rS[0:1, :])
                nc.gpsimd.partition_broadcast(bc_o[:], stg[0:1, :])
                nc.vector.tensor_tensor(
                    oT_sb[0:64, m, bass.ds(q0, 1024)],
                    oraw_e[:], bc_e[:], MULT)
                nc.vector.tensor_tensor(
                    oT_sb[64:128, m, bass.ds(q0, 1024)],
                    oraw_o[:], bc_o[:], MULT)

        if dump:
            d_qt = nc.dram_tensor("d_qt", [128, MT, S], BF16, kind="ExternalOutput").ap()
            d_kt = nc.dram_tensor("d_kt", [128, MT, S], BF16, kind="ExternalOutput").ap()
            d_v = nc.dram_tensor("d_v", [128, ST, NHL, HD], BF16, kind="ExternalOutput").ap()
            d_ot = nc.dram_tensor("d_ot", [128, MT, S], BF16, kind="ExternalOutput").ap()
            nc.sync.dma_start(d_qt, QT_sb[:])
            nc.sync.dma_start(d_kt, KT_sb[:])
            nc.sync.dma_start(d_v, V_sb[:])
            nc.sync.dma_start(d_ot, oT_sb[:])

        # ---- epilogue: out-projection second half (qt 8..15) ----
        for qt in range(8, ST):
            outproj_group(qt)


_CACHED = {}


def _get_bass():
    if "nc" not in _CACHED:
        _CACHED["nc"] = _build_bass()
    return _CACHED["nc"]


def _prep_core_inputs(c, query, key, value, Wq, bq, Wk, bk, Wv, bv, Wo):
    b, half = c // 2, c % 2
    sl = slice(DLOC * half, DLOC * half + DLOC)
    bq_sl = (bq[sl] * 0.125).astype(np.float32).reshape(MT, 128).T.copy()
    bk_sl = bk[sl].astype(np.float32).reshape(MT, 128).T.copy()
    return {
        "xqT": np.ascontiguousarray(query[b].T).astype(NPBF),
        "xkT": np.ascontiguousarray(key[b].T).astype(NPBF),
        "xvT": np.ascontiguousarray(value[b].T).astype(NPBF),
        "wq": np.ascontiguousarray(Wq[sl, :].T * 0.125).astype(NPBF),
        "wk": np.ascontiguousarray(Wk[sl, :].T).astype(NPBF),
        "wv": np.ascontiguousarray(Wv[sl, :].T).astype(NPBF),
        "wo": np.ascontiguousarray(Wo[:, sl].T).astype(NPBF),
        "bq": np.ascontiguousarray(bq_sl),
        "bk": np.ascontiguousarray(bk_sl),
        "bv": bv[sl].astype(np.float32).reshape(1, DLOC).copy(),
    }


def kernel(query, key, value, Wq, bq, Wk, bk, Wv, bv, Wo, bo,
           trace=False, **run_kwargs):
    query = np.asarray(query, np.float32)
    key = np.asarray(key, np.float32)
    value = np.asarray(value, np.float32)
    Wq, Wk, Wv, Wo = (np.asarray(w, np.float32) for w in (Wq, Wk, Wv, Wo))
    bq, bk, bv, bo = (np.asarray(x, np.float32) for x in (bq, bk, bv, bo))

    nc = _get_bass()
    in_maps = [_prep_core_inputs(c, query, key, value, Wq, bq, Wk, bk, Wv, bv, Wo)
               for c in range(8)]
    res = run_bass_kernel_spmd(nc, in_maps, core_ids=list(range(8)),
                               trace=trace, **run_kwargs)
    _CACHED["last_result"] = res

    B = query.shape[0]
    out = np.empty((B, S, E), np.float32)
    for b in range(B):
        out[b] = res.results[2 * b]["out"] + res.results[2 * b + 1]["out"] + bo
    return out
